# revision 5
# baseline (speedup 1.0000x reference)
"""Causal self-attention (B=2, T=2048, C=1024, H=16, D=64) on 8 trn2 cores.

Sharding: core c = (batch b = c//4, head-group g = c%4 covering heads 4g..4g+3).
QKV projection is column-parallel over the core's 12 head-channels blocks,
attention is fully local per head, output projection is row-parallel with the
partial sums reduced on the host (plus bproj).

Device dataflow (per core, all fp32):
  xT (C,T) @ wqkvT (C,768) -> qkv psum in (t, ch) layout, bias via K=1 matmul
  RoPE on q,k applied at psum->sbuf eviction (host permuted W rows so each
  head's channels are [evens(32) | odds(32)])
  PE-transpose q,k -> (d, t) layout
  S^T = kT.T @ qT per (head, 128-k-chunk, 512-q-chunk), causal-skipped
  P^T = exp(S^T * 0.125) on ACT, triangular-mask add on band tiles before exp
  yT' = v'.T @ P^T accumulated over k-chunks, where v' has a ones column so
        row 64 of yT' is the softmax denominator
  normalize: reciprocal + DMA partition-broadcast + DVE multiply
  out_partial = Y.T @ wprojT -> (t, n), DMA to DRAM
"""

import math

import numpy as np

import concourse.bass as bass
import concourse.mybir as mybir
from concourse.tile import TileContext
from concourse.bass_utils import run_bass_kernel_spmd

B, T, C, H = 2, 2048, 1024, 16
D = C // H  # 64
ROPE_BASE = 10000.0
N_CORES = 8
HPC = H // 4  # heads per core = 4
CPC = HPC * D  # channels per core = 256
TT = T // 128  # 16 t-tiles
NJ = T // 512  # 4 q-chunks
F32 = mybir.dt.float32

def _split_sync_waits(nc, cap=1):
    """This walrus build rejects instructions carrying more than `cap` sem
    waits; hoist the excess onto same-engine NoOp carriers placed just
    before the instruction."""
    ctr = 0
    for fn in nc.m.functions:
        for blk in fn.blocks:
            out = []
            for inst in blk.instructions:
                si = inst.sync_info
                if si is not None and si.on_wait and len(si.on_wait) > cap:
                    waits = list(si.on_wait)
                    rest, keep = waits[:-cap], waits[-cap:]
                    for k in range(0, len(rest), cap):
                        ctr += 1
                        nop = mybir.InstNoOp(
                            name=f"waitsplit-{ctr}", ins=[], outs=[]
                        )
                        nop.engine = inst.engine
                        nop.sync_info = mybir.SyncInfo(
                            on_wait=rest[k : k + cap], on_update=[]
                        )
                        nc.register_instruction(nop)
                        out.append(nop)
                    si.on_wait[:] = keep
                out.append(inst)
            blk.instructions[:] = out


def build_nc():
    nc = bass.Bass()

    xT = nc.dram_tensor("xT", [C, T], F32, kind="ExternalInput")
    wqkvT = nc.dram_tensor("wqkvT", [C, 3 * CPC], F32, kind="ExternalInput")
    bqkv_s = nc.dram_tensor("bqkv_s", [1, 3 * CPC], F32, kind="ExternalInput")
    wprojT = nc.dram_tensor("wprojT", [CPC, C], F32, kind="ExternalInput")
    cosr = nc.dram_tensor("cosr", [T, 128], F32, kind="ExternalInput")
    sinr = nc.dram_tensor("sinr", [T, 128], F32, kind="ExternalInput")
    trimask = nc.dram_tensor("trimask", [128, 128], F32, kind="ExternalInput")
    ident = nc.dram_tensor("ident", [128, 128], F32, kind="ExternalInput")
    out = nc.dram_tensor("out", [T, C], F32, kind="ExternalOutput")

    with TileContext(nc) as tc:
        with (
            tc.tile_pool(name="const", bufs=1) as cpool,
            tc.tile_pool(name="xin", bufs=3) as xpool,
            tc.tile_pool(name="qk", bufs=3) as qkpool,
            tc.tile_pool(name="pT", bufs=4) as ppool,
            tc.tile_pool(name="norm", bufs=3) as npool,
            tc.tile_pool(name="obuf", bufs=3) as opool,
            tc.tile_pool(name="persist", bufs=1) as perpool,
            tc.tile_pool(name="ps1", bufs=6, space="PSUM") as ps1,
            tc.tile_pool(name="psy", bufs=2, space="PSUM") as psy,
        ):
            # ---- constants / weights resident in SBUF ----
            wq_sb = cpool.tile([128, 8, 3 * CPC], F32, tag="wq")
            nc.sync.dma_start(wq_sb[:], wqkvT.rearrange("(kc p) n -> p kc n", p=128))
            wp_sb = cpool.tile([128, 2, C], F32, tag="wp")
            nc.sync.dma_start(wp_sb[:], wprojT.rearrange("(j p) n -> p j n", p=128))
            cos_sb = cpool.tile([128, TT, 128], F32, tag="cos")
            nc.sync.dma_start(cos_sb[:], cosr.rearrange("(tt p) f -> p tt f", p=128))
            sin_sb = cpool.tile([128, TT, 128], F32, tag="sin")
            nc.sync.dma_start(sin_sb[:], sinr.rearrange("(tt p) f -> p tt f", p=128))
            tri_sb = cpool.tile([128, 128], F32, tag="tri")
            nc.sync.dma_start(tri_sb[:], trimask[:, :])
            id_sb = cpool.tile([128, 128], F32, tag="id")
            nc.sync.dma_start(id_sb[:], ident[:, :])
            bq_sb = cpool.tile([1, 3 * CPC], F32, tag="bq")
            nc.sync.dma_start(bq_sb[:], bqkv_s[:, :])
            ones_sb = cpool.tile([1, 128], F32, tag="ones")
            nc.vector.memset(ones_sb[:], 1.0)

            # persistent activations
            v_sb = perpool.tile([128, TT, HPC * (D + 1)], F32, tag="v")
            qT_sb = perpool.tile([128, 2, TT, 128], F32, tag="qT")
            kT_sb = perpool.tile([128, 2, TT, 128], F32, tag="kT")
            y_sb = [
                perpool.tile([128, T], F32, tag=f"y{j}", name=f"y_sb{j}")
                for j in range(2)
            ]

            # ---- phase 1: qkv projection + rope + transpose ----
            for tt in range(TT):
                xt = xpool.tile([128, 8, 128], F32, tag="xt")
                nc.sync.dma_start(
                    xt[:],
                    xT.rearrange("(kc p) t -> p kc t", p=128)[
                        :, :, tt * 128 : (tt + 1) * 128
                    ],
                )
                qk_ps = ps1.tile([128, 512], F32, tag="ps1")  # q(256) | k(256)
                v_ps = ps1.tile([128, 256], F32, tag="ps1")  # v(256)
                for kc in range(8):
                    nc.tensor.matmul(
                        qk_ps[:],
                        lhsT=xt[:, kc, :],
                        rhs=wq_sb[:, kc, 0:512],
                        start=(kc == 0),
                        stop=False,
                    )
                    nc.tensor.matmul(
                        v_ps[:],
                        lhsT=xt[:, kc, :],
                        rhs=wq_sb[:, kc, 512:768],
                        start=(kc == 0),
                        stop=False,
                    )
                nc.tensor.matmul(
                    qk_ps[:], lhsT=ones_sb[:], rhs=bq_sb[:, 0:512],
                    start=False, stop=True,
                )
                nc.tensor.matmul(
                    v_ps[:], lhsT=ones_sb[:], rhs=bq_sb[:, 512:768],
                    start=False, stop=True,
                )

                # rope eviction for q and k: per half (evens A / odds B):
                #   A' = A*cos - B*sin ; B' = B*cos + A*sin
                qs = qkpool.tile([128, 512], F32, tag="qs")
                cos_t = cos_sb[:, tt, :].rearrange("p (h f) -> p h f", f=32)
                sin_t = sin_sb[:, tt, :].rearrange("p (h f) -> p h f", f=32)
                for half in range(2):  # 0: q cols 0:256, 1: k cols 256:512
                    base = qk_ps[:, half * 256 : half * 256 + 256].rearrange(
                        "p (h d) -> p h d", d=D
                    )
                    dst = qs[:, half * 256 : half * 256 + 256].rearrange(
                        "p (h d) -> p h d", d=D
                    )
                    A, Bo = base[:, :, 0:32], base[:, :, 32:64]
                    Ad, Bd = dst[:, :, 0:32], dst[:, :, 32:64]
                    tmp = qkpool.tile([128, 2, 4, 32], F32, tag="ropetmp")
                    nc.vector.tensor_mul(Ad, A, cos_t)
                    nc.vector.tensor_mul(tmp[:, 0], Bo, sin_t)
                    nc.vector.tensor_sub(Ad, Ad, tmp[:, 0])
                    nc.vector.tensor_mul(Bd, Bo, cos_t)
                    nc.vector.tensor_mul(tmp[:, 1], A, sin_t)
                    nc.vector.tensor_add(Bd, Bd, tmp[:, 1])

                # v eviction (strided dest leaves a ones column per head)
                nc.scalar.copy(
                    out=v_sb[:, tt, :].rearrange("p (h e) -> p h e", e=D + 1)[
                        :, :, 0:D
                    ],
                    in_=v_ps[:].rearrange("p (h d) -> p h d", d=D),
                )
                nc.vector.memset(
                    v_sb[:, tt, :].rearrange("p (h e) -> p h e", e=D + 1)[:, :, D : D + 1],
                    1.0,
                )

                # transpose q,k head-pairs into (d, t) layout
                for j in range(2):
                    tp = ps1.tile([128, 128], F32, tag="ps1")
                    nc.tensor.transpose(tp[:], qs[:, j * 128 : (j + 1) * 128], id_sb[:])
                    nc.any.tensor_copy(qT_sb[:, j, tt, :], tp[:])
                    tp2 = ps1.tile([128, 128], F32, tag="ps1")
                    nc.tensor.transpose(
                        tp2[:], qs[:, 256 + j * 128 : 256 + (j + 1) * 128], id_sb[:]
                    )
                    nc.any.tensor_copy(kT_sb[:, j, tt, :], tp2[:])

            # ---- phase 2: attention per (q-chunk J, head h) ----
            for J in range(NJ):
                for h in range(HPC):
                    j, po = h // 2, 64 * (h % 2)
                    yp = psy.tile([128, 512], F32, tag="psy")
                    nlast = 4 * J + 3
                    for i in range(4 * J + 4):
                        r = i - 4 * J  # band index when >= 0
                        n0 = 128 * r if r > 0 else 0
                        sp = ps1.tile([128, 512], F32, tag="ps1")
                        nc.tensor.matmul(
                            sp[:, n0:512],
                            lhsT=kT_sb[po : po + 64, j, i, :],
                            rhs=qT_sb[po : po + 64, j, 4 * J : 4 * J + 4, :]
                            .rearrange("p a b -> p (a b)")[:, n0:512],
                            start=True,
                            stop=True,
                        )
                        if r >= 0:
                            nc.vector.tensor_add(
                                sp[:, n0 : n0 + 128], sp[:, n0 : n0 + 128], tri_sb[:]
                            )
                        pT = ppool.tile([128, 512], F32, tag="pT")
                        nc.scalar.activation(
                            pT[:, n0:512],
                            sp[:, n0:512],
                            mybir.ActivationFunctionType.Exp,
                            scale=1.0 / math.sqrt(D),
                        )
                        nc.tensor.matmul(
                            yp[0:65, n0:512],
                            lhsT=v_sb[:, i, h * (D + 1) : (h + 1) * (D + 1)],
                            rhs=pT[:, n0:512],
                            start=(i == 0),
                            stop=(i == nlast),
                        )
                    # normalize: y[d, q] * (1 / y[64, q])
                    yts = npool.tile([65, 512], F32, tag="yts")
                    nc.scalar.copy(out=yts[:], in_=yp[0:65, :])
                    rec = npool.tile([1, 512], F32, tag="rec")
                    nc.vector.reciprocal(rec[:], yts[64:65, :])
                    rb = ps1.tile([64, 512], F32, tag="ps1")
                    nc.tensor.matmul(
                        rb[:], lhsT=ones_sb[:, 0:64], rhs=rec[:],
                        start=True, stop=True,
                    )
                    nc.vector.tensor_mul(
                        y_sb[j][po : po + 64, J * 512 : (J + 1) * 512],
                        yts[0:64, :],
                        rb[:],
                    )

            # ---- phase 3: output projection (partial; host reduces) ----
            for tt in range(TT):
                for nn in range(2):
                    op = ps1.tile([128, 512], F32, tag="ps1")
                    for j in range(2):
                        nc.tensor.matmul(
                            op[:],
                            lhsT=y_sb[j][:, tt * 128 : (tt + 1) * 128],
                            rhs=wp_sb[:, j, nn * 512 : (nn + 1) * 512],
                            start=(j == 0),
                            stop=(j == 1),
                        )
                    ob = opool.tile([128, 512], F32, tag="ob")
                    nc.any.tensor_copy(ob[:], op[:])
                    nc.sync.dma_start(
                        out[tt * 128 : (tt + 1) * 128, nn * 512 : (nn + 1) * 512],
                        ob[:],
                    )
    _split_sync_waits(nc)
    return nc


_nc_cache = None


def _get_nc():
    global _nc_cache
    if _nc_cache is None:
        _nc_cache = build_nc()
    return _nc_cache


_PERM = np.concatenate([np.arange(0, D, 2), np.arange(1, D, 2)])  # [evens|odds]


def make_inputs(x, Wqkv, bqkv, Wproj):
    """Host-side sharding: returns list of 8 per-core input dicts."""
    theta = np.exp(
        np.arange(0, D, 2, dtype=np.float64) * (-math.log(ROPE_BASE) / D)
    )
    ang = np.arange(T, dtype=np.float64)[:, None] * theta[None, :]
    cosr = np.tile(np.cos(ang), (1, 4)).astype(np.float32)
    sinr = np.tile(np.sin(ang), (1, 4)).astype(np.float32)
    cosr = np.ascontiguousarray(cosr)
    sinr = np.ascontiguousarray(sinr)
    kk, qq = np.meshgrid(np.arange(128), np.arange(128), indexing="ij")
    trimask = np.where(qq >= kk, 0.0, -1e30).astype(np.float32)
    ident = np.eye(128, dtype=np.float32)

    in_maps = []
    for c in range(N_CORES):
        b, g = divmod(c, 4)
        heads = range(4 * g, 4 * g + 4)
        rows = []
        for part in range(3):  # q, k, v blocks of Wqkv
            for h in heads:
                blk = np.arange(part * C + h * D, part * C + (h + 1) * D)
                rows.append(blk[_PERM] if part < 2 else blk)
        rows = np.concatenate(rows)
        W_s = Wqkv[rows]  # (768, 1024)
        in_maps.append(
            {
                "xT": np.ascontiguousarray(x[b].T),
                "wqkvT": np.ascontiguousarray(W_s.T),
                "bqkv_s": np.ascontiguousarray(bqkv[rows][None, :]),
                "wprojT": np.ascontiguousarray(
                    Wproj[:, 256 * g : 256 * (g + 1)].T
                ),
                "cosr": cosr,
                "sinr": sinr,
                "trimask": trimask,
                "ident": ident,
            }
        )
    return in_maps


def kernel(x, Wqkv, bqkv, Wproj, bproj):
    x = np.asarray(x, dtype=np.float32)
    Wqkv = np.asarray(Wqkv, dtype=np.float32)
    bqkv = np.asarray(bqkv, dtype=np.float32)
    Wproj = np.asarray(Wproj, dtype=np.float32)
    bproj = np.asarray(bproj, dtype=np.float32)

    nc = _get_nc()
    in_maps = make_inputs(x, Wqkv, bqkv, Wproj)
    res = run_bass_kernel_spmd(nc, in_maps, core_ids=list(range(N_CORES)))
    out = np.zeros((B, T, C), dtype=np.float32)
    for c in range(N_CORES):
        out[c // 4] += res.results[c]["out"]
    out += bproj[None, None, :]
    return out
